# revision 38
# baseline (speedup 1.0000x reference)
"""Trainium2 Bass kernel for nn_Block_20289425506613 (MLA transformer block).

Sharding: 8 cores = 4 batches x 2 query-halves (interleaved time blocks for
causal load balance). No collectives: each core recomputes full-batch K/V.
fp8e4m3 DoubleRow matmuls for all attention-side projections and P·V
(host-scaled weights, descale fused into PSUM eviction); FFN stays bf16.
"""
import sys

for _p in ("/opt/trn_rl_repo", "/root/.axon_site/_ro/trn_rl_repo"):
    if _p not in sys.path:
        sys.path.insert(0, _p)

import numpy as np

# ---------- constants (hardcoded per problem spec) ----------
B, T, C = 4, 1024, 1024
NH, L, DHR, FF = 16, 512, 64, 4096
DK = C // NH  # 64
TWO_L = 2 * L  # 1024
EPS = 1e-6
NEG = -9e15
SCALE = 1.0 / np.sqrt(DK)  # 0.125, folded into W_q / W_qr on host
P = 128
TQ = 512            # query tokens per core
NCORES = 8
KT_C = C // P       # 8
KT_L = L // P       # 4
KT_FF = FF // P     # 32
NPAIR = NH // 2     # 8 head-pair tiles
NKB = T // P        # 8 key blocks

S_W = 32.0          # fp8 weight scale (w_dkv, w_kr, w_kv, w_o)
S_Q = 256.0         # fp8 weight scale for W_q / W_qr (after folding SCALE)

_CACHE = {}


# ---------- bass program ----------
def _build_program(repeat=None, skip=()):
    import concourse.bass as bass
    from concourse import bacc, tile, mybir

    dt = mybir.dt
    AF = mybir.ActivationFunctionType
    DR = mybir.MatmulPerfMode.DoubleRow

    nc = bacc.Bacc("TRN2", target_bir_lowering=False, debug=False,
                   num_devices=NCORES)

    def din(name, shape, d=dt.float32):
        return nc.dram_tensor(name, list(shape), d, kind="ExternalInput").ap()

    f32, f32r, bf16, f8 = dt.float32, dt.float32r, dt.bfloat16, dt.float8e4

    xT_full = din("xT_full", [C, T])
    xT_sel = din("xT_sel", [C, TQ])
    w_dkv = din("w_dkv", [TWO_L // P, P, KT_C, P], f8)
    b_dkv = din("b_dkv", [P, KT_C])
    w_kr = din("w_kr", [P, KT_C, DHR], f8)
    b_kr = din("b_kr", [DHR, 1])
    w_qr = din("w_qr", [C // P, P, KT_C, P], f8)
    b_qr = din("b_qr", [P, KT_C])
    w_kv_k = din("w_kv_k", [C // P, P, KT_L, P], f8)
    w_kv_v = din("w_kv_v", [P, C // 512, KT_L, 512], f8)
    b_k = din("b_k", [P, KT_C])
    w_q = din("w_q", [C // P, P, KT_L, P], f8)
    b_q = din("b_q", [P, KT_C])
    w_o = din("w_o", [C // P, P, KT_C, P], f8)
    b_o = din("b_o", [P, KT_C])
    w_f1 = din("w_f1", [FF // P, P, KT_C * P], bf16)
    b_f1 = din("b_f1", [P, KT_FF])
    w_f2 = din("w_f2", [C // P, P, KT_FF * P], bf16)
    b_f2 = din("b_f2", [P, KT_C])
    cosk = din("cosk", [DHR, T], bf16)
    sink = din("sink", [DHR, T], bf16)
    cosq = din("cosq", [KT_C, P, TQ], bf16)
    sinq = din("sinq", [KT_C, P, TQ], bf16)
    amask01 = din("amask01", [NKB, P, 2 * P], f8)
    ones128 = din("ones128", [P, P], f32r)
    rot128 = din("rot128", [P, P], f32r)
    selpair = din("selpair", [2, P], f32r)
    id128 = din("id128", [P, P], bf16)
    epsc = din("epsc", [P, 1])

    outT = nc.dram_tensor("outT", [C, TQ], f32, kind="ExternalOutput").ap()

    HALF = 512

    with tile.TileContext(nc) as tc:
        from contextlib import ExitStack
        est = ExitStack()
        with est:
            if repeat is not None:
                est.enter_context(tc.For_i(0, repeat, 1))
            constp = est.enter_context(tc.tile_pool(name="const", bufs=1))
            attnp = est.enter_context(tc.tile_pool(name="attn", bufs=1))
            work = est.enter_context(tc.tile_pool(name="work", bufs=2))
            psum = est.enter_context(tc.tile_pool(name="psum", bufs=2, space="PSUM"))
            psacc = est.enter_context(tc.tile_pool(name="psacc", bufs=1, space="PSUM"))
            xselp = est.enter_context(tc.tile_pool(name="xselp", bufs=1))

            def cload(pool, shape, dram_ap, d=f32, tag="c", eng=None):
                t = pool.tile(shape, d, tag=tag, name=tag)
                (eng or nc.sync).dma_start(t[:], dram_ap)
                return t

            OPM, OPA = mybir.AluOpType.mult, mybir.AluOpType.add

            # tiny consts that gate the first matmuls go first
            ones_sb = cload(constp, [P, P], ones128, f32r, "ones", nc.gpsimd)
            eps_sb = cload(constp, [P, 1], epsc, f32, "eps", nc.gpsimd)

            # x loads next — they gate the rmsnorm/comp pipeline
            x_sel = xselp.tile([P, KT_C, TQ], f32, name="x_sel")
            xsr = xT_sel.rearrange("(kt p) t -> p kt t", p=P)
            compp_ctx = tc.tile_pool(name="compp", bufs=1)
            compp = compp_ctx.__enter__()
            xnp_ctx = tc.tile_pool(name="xnp", bufs=1)
            xnp = xnp_ctx.__enter__()
            x_full = xnp.tile([P, KT_C, T], f32, tag="xfx", name="xfx")
            xfr = xT_full.rearrange("(kt p) t -> p kt t", p=P)
            rot3 = (nc.gpsimd, nc.scalar, nc.sync)
            for kt in range(KT_C):
                rot3[kt % 3].dma_start(x_full[:, kt], xfr[:, kt])
                rot3[(kt + 1) % 3].dma_start(x_sel[:, kt], xsr[:, kt])

            selpair_sb = cload(constp, [2, P], selpair, f32r, "selp2")
            id_sb = cload(constp, [P, P], id128, bf16, "id128")
            bdkv_sb = cload(constp, [P, KT_C], b_dkv, f32, "bdkv")
            bkr_sb = cload(constp, [DHR, 1], b_kr, f32, "bkr")
            bqr_sb = cload(constp, [P, KT_C], b_qr, f32, "bqr")
            bk_sb = cload(constp, [P, KT_C], b_k, f32, "bk")
            bq_sb = cload(constp, [P, KT_C], b_q, f32, "bq")
            bo_sb = cload(constp, [P, KT_C], b_o, f32, "bo")
            bf1_sb = cload(constp, [P, KT_FF], b_f1, f32, "bf1")
            bf2_sb = cload(constp, [P, KT_C], b_f2, f32, "bf2")
            rot_sb = cload(constp, [P, P], rot128, f32r, "rot")
            cosk_sb = cload(constp, [DHR, T], cosk, bf16, "cosk")
            sink_sb = cload(constp, [DHR, T], sink, bf16, "sink")
            wkr_sb = cload(constp, [P, KT_C, DHR], w_kr, f8, "wkr")
            am_sb = cload(constp, [P, NKB, 2 * P],
                          amask01.rearrange("kb p t -> p kb t"), f8, "am")

            # rms-normalized fp8 copy of x; squares on Pool/DVE,
            # partition-reduction via ones-matmul on PE.
            def rmsnorm8(pool, x_sb, W, tag):
                xn8 = pool.tile([P, KT_C, W], f8, tag=tag + "n", name=tag + "n")
                for ch in range(W // HALF):
                    sl = slice(ch * HALF, (ch + 1) * HALF)
                    ps = psum.tile([P, HALF], f32, tag="ps", name="ps")
                    for kt in range(KT_C):
                        sq = work.tile([P, HALF], f32r, tag="sq", name="sq")
                        (nc.vector if kt % 2 else nc.gpsimd).tensor_mul(
                            sq[:], x_sb[:, kt, sl], x_sb[:, kt, sl])
                        nc.tensor.matmul(ps[:], ones_sb[:], sq[:],
                                         start=(kt == 0), stop=(kt == KT_C - 1))
                    rstd = work.tile([P, HALF], f32, tag="rstd", name="rstd")
                    nc.scalar.activation(rstd[:], ps[:], AF.Sqrt, bias=eps_sb[:], scale=1.0 / C)
                    nc.vector.reciprocal(rstd[:], rstd[:])
                    for kt in range(KT_C):
                        (nc.vector if kt % 2 else nc.gpsimd).tensor_mul(
                            xn8[:, kt, sl], x_sb[:, kt, sl], rstd[:])
                return xn8

            # per-head tiles on 64 partitions: slab 0 = k/q head dims, slab 1 = rope
            kcat = attnp.tile([DK, 2, NH, T], f8, name="kcat")
            qcat = attnp.tile([DK, 2, NH, TQ], f8, name="qcat")

            # evict a [128, W] psum holding a head PAIR into cat rows 0:64
            # even head -> direct evict; odd head -> stage rows 64:128, DMA shift
            def evict_pair(ps_ap, nt, sl, cat, bias, stage_pool, W, scl,
                           eng_dma, eng_ev):
                stg = stage_pool.tile([P, W], f8, tag="stg", name="stg")
                if eng_ev is nc.scalar:
                    nc.scalar.activation(stg[:], ps_ap[:], AF.Identity,
                                         bias=bias[:, nt:nt + 1], scale=scl)
                else:
                    eng_ev.tensor_scalar(stg[:], ps_ap[:], scl,
                                         bias[:, nt:nt + 1], OPM, OPA)
                nc.vector.tensor_copy(cat[:, 0, 2 * nt, sl], stg[0:DK, :])
                eng_dma.dma_start(cat[:, 0, 2 * nt + 1, sl], stg[DK:P, :])

            # ============ phase A+B head: rmsnorms + combined comp dense ============
            comp_sel = compp.tile([P, KT_C, TQ], f8, name="comp_sel")
            comp_full = compp.tile([P, KT_C, T], f8, name="comp_full")
            with tc.tile_pool(name="wsel", bufs=2) as wsel:
                xn8f = rmsnorm8(xnp, x_full, T, "xf")
                xn8s = rmsnorm8(xselp, x_sel, TQ, "xs")
                for nt in range(KT_C):
                    wt = wsel.tile([P, KT_C, P], f8, tag="w8", name="wt")
                    nc.sync.dma_start(wt[:], w_dkv[nt])
                    for si, (src_x, dst, sl) in enumerate((
                        (xn8f, comp_full, slice(0, HALF)),
                        (xn8f, comp_full, slice(HALF, T)),
                        (xn8s, comp_sel, slice(0, TQ)),
                    )):
                        ps = psum.tile([P, HALF], f32, tag="ps", name="ps")
                        for i in range(0, KT_C, 2):
                            nc.tensor.matmul(ps[:], wt[:, i:i + 2], src_x[:, i:i + 2, sl],
                                             start=(i == 0), stop=(i == KT_C - 2),
                                             perf_mode=DR)
                        if (nt + si) % 2:
                            nc.vector.tensor_scalar(
                                dst[:, nt, sl], ps[:], 1.0 / S_W,
                                bdkv_sb[:, nt:nt + 1], OPM, OPA)
                        else:
                            nc.scalar.activation(
                                dst[:, nt, sl], ps[:], AF.Identity,
                                bias=bdkv_sb[:, nt:nt + 1], scale=1.0 / S_W)
            xnp_ctx.__exit__(None, None, None)

            # ============ phase A tail: q/qR -> qcat ============
            with tc.tile_pool(name="wsel4", bufs=3) as wsel4, \
                 tc.tile_pool(name="stgA", bufs=2) as stgA:
                # q -> qcat rows 0:64
                for nt in range(KT_C):
                    wt = wsel4.tile([P, KT_L, P], f8, tag="w4", name="wt")
                    nc.sync.dma_start(wt[:], w_q[nt])
                    ps = psum.tile([P, TQ], f32, tag="ps", name="ps")
                    for i in range(0, KT_L, 2):
                        nc.tensor.matmul(ps[:], wt[:, i:i + 2],
                                         comp_sel[:, KT_L + i:KT_L + i + 2],
                                         start=(i == 0), stop=(i == KT_L - 2),
                                         perf_mode=DR)
                    evict_pair(ps, nt, slice(0, TQ), qcat, bq_sb, stgA, TQ,
                               1.0 / S_Q, nc.sync,
                               (nc.vector, nc.scalar)[nt % 2])
                # qR -> qcat rows 64:128
                for nt in range(KT_C):
                    wt = wsel4.tile([P, KT_C, P], f8, tag="w8b", name="wt")
                    nc.scalar.dma_start(wt[:], w_qr[nt])
                    ps = psum.tile([P, TQ], f32, tag="ps", name="ps")
                    for i in range(0, KT_C, 2):
                        nc.tensor.matmul(ps[:], wt[:, i:i + 2], comp_sel[:, i:i + 2],
                                         start=(i == 0), stop=(i == KT_C - 2),
                                         perf_mode=DR)
                    qn = work.tile([P, TQ], f32r, tag="sq", name="qn")
                    nc.scalar.activation(qn[:], ps[:], AF.Identity,
                                         bias=bqr_sb[:, nt:nt + 1], scale=1.0 / S_Q)
                    cq = stgA.tile([P, TQ], bf16, tag="cq", name="cq")
                    nc.scalar.dma_start(cq[:], cosq[nt])
                    sq = stgA.tile([P, TQ], bf16, tag="sqr", name="sq")
                    nc.scalar.dma_start(sq[:], sinq[nt])
                    psr = psum.tile([P, TQ], f32, tag="ps", name="psr")
                    nc.tensor.matmul(psr[:], rot_sb[:], qn[:], start=True, stop=True)
                    t1 = work.tile([P, TQ], f32, tag="t1", name="t1")
                    nc.vector.tensor_mul(t1[:], qn[:].bitcast(f32), cq[:])
                    t2 = work.tile([P, TQ], f32, tag="t2", name="t2")
                    nc.vector.tensor_mul(t2[:], psr[:], sq[:])
                    qrope = stgA.tile([P, TQ], f8, tag="stg", name="qrope")
                    nc.vector.tensor_add(qrope[:], t1[:], t2[:])
                    nc.vector.tensor_copy(qcat[:, 1, 2 * nt, :], qrope[0:DK, :])
                    nc.sync.dma_start(qcat[:, 1, 2 * nt + 1, :], qrope[DK:P, :])

            # ============ phase B: full path -> kcat, v2 ============
            with tc.tile_pool(name="fullp2", bufs=1) as fullp2:
                v2 = attnp.tile([P, NKB, NH * 65], f8, name="v2")
                nc.any.memset(
                    v2[:].rearrange("p kb (h c) -> p kb h c", c=65)[:, :, :, 64:65], 1.0)
                # kR -> kcat rows 64:128 (same for every head)
                wkvp_ctx = tc.tile_pool(name="wkvp", bufs=3)
                wkvp = wkvp_ctx.__enter__()
                with tc.tile_pool(name="krp", bufs=1) as krp:
                    kr_raw = krp.tile([DHR, T], f32r, name="kr_raw")
                    krr = krp.tile([DHR, T], f8, name="krr")
                    for ch in range(T // HALF):
                        sl = slice(ch * HALF, (ch + 1) * HALF)
                        ps = psum.tile([DHR, HALF], f32, tag="ps", name="ps")
                        for kt in range(0, KT_C, 2):
                            nc.tensor.matmul(ps[:], wkr_sb[:, kt:kt + 2],
                                             comp_full[:, kt:kt + 2, sl],
                                             start=(kt == 0), stop=(kt == KT_C - 2),
                                             perf_mode=DR)
                        nc.vector.tensor_scalar(kr_raw[:, sl], ps[:], 1.0 / S_W,
                                                bkr_sb[:], OPM, OPA)
                    for ch in range(T // HALF):
                        sl = slice(ch * HALF, (ch + 1) * HALF)
                        psr = psum.tile([DHR, HALF], f32, tag="ps", name="psr")
                        nc.tensor.matmul(psr[:], rot_sb[:DHR, :DHR], kr_raw[:, sl],
                                         start=True, stop=True)
                        t1 = work.tile([DHR, HALF], f32, tag="t1", name="t1")
                        nc.vector.tensor_mul(t1[:], kr_raw[:, sl].bitcast(f32), cosk_sb[:, sl])
                        t2 = work.tile([DHR, HALF], f32, tag="t2", name="t2")
                        nc.vector.tensor_mul(t2[:], psr[:], sink_sb[:, sl])
                        nc.vector.tensor_add(krr[:, sl], t1[:], t2[:])
                    engs = (nc.scalar, nc.gpsimd, nc.sync)
                    for h in range(NH):
                        engs[h % 3].dma_start(kcat[:, 1, h, :], krr[:, :])
                # k -> kcat rows 0:64 ; v -> v2
                with tc.tile_pool(name="stgB", bufs=2) as stgB:
                    for nt in range(KT_C):
                        wt = wkvp.tile([P, KT_L, P], f8, tag="w4", name="wt")
                        nc.sync.dma_start(wt[:], w_kv_k[nt])
                        for ch in range(T // HALF):
                            sl = slice(ch * HALF, (ch + 1) * HALF)
                            ps = psum.tile([P, HALF], f32, tag="ps", name="ps")
                            for i in range(0, KT_L, 2):
                                nc.tensor.matmul(ps[:], wt[:, i:i + 2],
                                                 comp_full[:, i:i + 2, sl],
                                                 start=(i == 0), stop=(i == KT_L - 2),
                                                 perf_mode=DR)
                            evict_pair(ps, nt, sl, kcat, bk_sb, stgB, HALF,
                                       1.0 / S_W, (nc.sync, nc.scalar)[nt % 2],
                                       nc.vector)
                    wv_sb = fullp2.tile([P, 2, KT_L, HALF], f8, name="wv_sb")
                    nc.sync.dma_start(wv_sb[:], w_kv_v)
                    for kb in range(NKB):
                        for ch in range(C // HALF):
                            sl = slice(ch * HALF, (ch + 1) * HALF)
                            ps = psum.tile([P, HALF], f32, tag="ps", name="ps")
                            for lt in range(0, KT_L, 2):
                                nc.tensor.matmul(
                                    ps[:],
                                    comp_full[:, lt:lt + 2, kb * P:(kb + 1) * P],
                                    wv_sb[:, ch, lt:lt + 2],
                                    start=(lt == 0), stop=(lt == KT_L - 2),
                                    perf_mode=DR)
                            dst = v2[:, kb, ch * 8 * 65:(ch + 1) * 8 * 65]
                            dst = dst.rearrange("p (h c) -> p h c", c=65)[:, :, 0:64]
                            psv = ps[:].rearrange("p (h c) -> p h c", c=64)
                            if (kb + ch) % 2:
                                nc.vector.tensor_scalar_mul(dst, psv, 1.0 / S_W)
                            else:
                                nc.scalar.mul(dst, psv, 1.0 / S_W)
                wkvp_ctx.__exit__(None, None, None)
            compp_ctx.__exit__(None, None, None)

            # ============ phase C: attention ============
            postp = est.enter_context(tc.tile_pool(name="postp", bufs=1))
            wop_ctx = tc.tile_pool(name="wop", bufs=3)
            wop = wop_ctx.__enter__()
            o_sb = postp.tile([P, NPAIR, TQ], f8, name="o_sb")
            with tc.tile_pool(name="amp", bufs=1) as amp, \
                 tc.tile_pool(name="cwork", bufs=3) as cwork:
                if "C" in skip:
                    nc.any.memset(o_sb[:], 0.001)

                # normalization is software-pipelined one pair behind the
                # matmuls so the selpair-broadcast matmul never stalls PE
                def norm_pair(r, osg, rd2):
                    psb = psum.tile([P, TQ], f32, tag="ps", name="psb")
                    nc.tensor.matmul(psb[:], selpair_sb[:], rd2[:],
                                     start=True, stop=True)
                    nc.vector.tensor_mul(o_sb[:, r], osg[:], psb[:])

                prev = None
                def pv_step(r, kbp, es8, pv_e, pv_o):
                    qs0 = (0 if kbp == 0 else kbp) * P
                    st = kbp == 0
                    last = kbp == NKB // 2 - 1
                    for e in range(2):
                        h = 2 * r + e
                        pv = pv_e if e == 0 else pv_o
                        vsl = v2[:, 2 * kbp:2 * kbp + 2, h * 65:h * 65 + 65]
                        nc.tensor.matmul(pv[:, qs0:TQ], vsl,
                                         es8[:, e, :, qs0:TQ],
                                         start=st, stop=last, perf_mode=DR)

                for r in [] if "C" in skip else range(NPAIR):
                    pv_e = psacc.tile([65, TQ], f32, tag="pv_e", name="pv_e")
                    pv_o = psacc.tile([65, TQ], f32, tag="pv_o", name="pv_o")
                    for kbp in range(NKB // 2):
                        j0 = max(0, -(-(2 * kbp - 1) // 2))
                        qs0 = j0 * P
                        wdt = TQ - qs0
                        es8 = cwork.tile([P, 2, 2, TQ], f8, tag="es", name="es")
                        for kbi in range(2):
                            kb = 2 * kbp + kbi
                            kslc = slice(kb * P, (kb + 1) * P)
                            qsl = slice(qs0, TQ)
                            ssp = psum.tile([P, 2, TQ], f32, tag="ssp", name="ssp")
                            jm = kbp
                            for e in range(2):
                                h = 2 * r + e
                                nc.tensor.matmul(ssp[:, e, qsl], kcat[:, :, h, kslc],
                                                 qcat[:, :, h, qsl],
                                                 start=True, stop=True, perf_mode=DR)
                            nc.scalar.activation(
                                es8[:, :, kbi, qs0:], ssp[:, :, qs0:], AF.Exp)
                            dg = slice(jm * P, (jm + 1) * P)
                            nc.gpsimd.tensor_mul(
                                es8[:, :, kbi, dg], es8[:, :, kbi, dg],
                                am_sb[:, kb].rearrange("p (e c) -> p e c", e=2))
                        pv_step(r, kbp, es8, pv_e, pv_o)
                    osg = cwork.tile([P, TQ], f32r, tag="osg", name="osg")
                    nc.vector.tensor_copy(osg[0:65, :], pv_e[0:65])
                    ot_o = cwork.tile([65, TQ], f32r, tag="ot", name="ot_o")
                    nc.vector.tensor_copy(ot_o[:], pv_o[:])
                    rd2 = cwork.tile([2, TQ], f32r, tag="rd2", name="rd2")
                    nc.sync.dma_start(rd2[0:1, :], osg[64:65])
                    nc.gpsimd.dma_start(rd2[1:2, :], ot_o[64:65])
                    nc.sync.dma_start(osg[DK:P, :], ot_o[0:DK])
                    with nc.allow_low_precision(reason="f32r 1/d is benign"):
                        nc.vector.reciprocal(rd2[:], rd2[:])
                    if prev is not None:
                        norm_pair(*prev)
                    prev = (r, osg, rd2)
                if prev is not None:
                    norm_pair(*prev)

            # ============ phase D: normalize + W_o + residual ============
            h1 = postp.tile([P, KT_C, TQ], f32, name="h1")
            for nt in range(KT_C):
                wt = wop.tile([P, KT_C, P], f8, tag="wo", name="wt")
                nc.sync.dma_start(wt[:], w_o[nt])
                ps = psum.tile([P, TQ], f32, tag="ps", name="ps")
                for r in range(0, NPAIR, 2):
                    nc.tensor.matmul(ps[:], wt[:, r:r + 2], o_sb[:, r:r + 2],
                                     start=(r == 0), stop=(r == NPAIR - 2),
                                     perf_mode=DR)
                tb = work.tile([P, TQ], f32, tag="t1", name="tb")
                nc.scalar.activation(tb[:], ps[:], AF.Identity,
                                     bias=bo_sb[:, nt:nt + 1], scale=1.0 / S_W)
                nc.vector.tensor_add(h1[:, nt], tb[:], x_sel[:, nt])
            wop_ctx.__exit__(None, None, None)

            # ============ phase E: FFN ============
            if "E" in skip:
                for nt in range(KT_C):
                    nc.sync.dma_start(outT.rearrange("(kt p) t -> p kt t", p=P)[:, nt], h1[:, nt])
            with tc.tile_pool(name="ffnp", bufs=1) as ffnp, \
                 tc.tile_pool(name="wffn", bufs=3) as wffn:
                if "E" not in skip:
                    ps = psum.tile([P, TQ], f32, tag="ps", name="ps")
                    for kt in range(KT_C):
                        sq = work.tile([P, TQ], f32r, tag="sq", name="sq")
                        nc.gpsimd.tensor_mul(sq[:], h1[:, kt], h1[:, kt])
                        nc.tensor.matmul(ps[:], ones_sb[:], sq[:],
                                         start=(kt == 0), stop=(kt == KT_C - 1))
                    rstd2 = work.tile([P, TQ], f32, tag="rstd", name="rstd2")
                    nc.scalar.activation(rstd2[:], ps[:], AF.Sqrt, bias=eps_sb[:], scale=1.0 / C)
                    nc.vector.reciprocal(rstd2[:], rstd2[:])
                    h1n = ffnp.tile([P, KT_C, TQ], bf16, name="h1n")
                    for kt in range(KT_C):
                        (nc.vector if kt % 2 else nc.gpsimd).tensor_mul(
                            h1n[:, kt], h1[:, kt], rstd2[:])
                    g_sb = ffnp.tile([P, KT_FF, TQ], bf16, name="g_sb")
                    engs1 = (nc.sync, nc.scalar, nc.gpsimd)
                    for nt in range(KT_FF):
                        wt = wffn.tile([P, KT_C * P], bf16, tag="wf1", name="wt")
                        engs1[nt % 3].dma_start(wt[:], w_f1[nt])
                        ps = psum.tile([P, TQ], f32, tag="ps", name="ps")
                        for i in range(KT_C):
                            nc.tensor.matmul(ps[:], wt[:, i * P:(i + 1) * P], h1n[:, i],
                                             start=(i == 0), stop=(i == KT_C - 1))
                        nc.scalar.activation(g_sb[:, nt], ps[:], AF.Gelu_apprx_tanh,
                                             bias=bf1_sb[:, nt:nt + 1])
                    for nt in range(KT_C):
                        wt = wffn.tile([P, KT_FF * P], bf16, tag="wf2", name="wt")
                        engs1[nt % 3].dma_start(wt[:], w_f2[nt])
                        ps = psum.tile([P, TQ], f32, tag="ps", name="ps")
                        for kt in range(KT_FF):
                            nc.tensor.matmul(ps[:], wt[:, kt * P:(kt + 1) * P], g_sb[:, kt],
                                             start=(kt == 0), stop=(kt == KT_FF - 1))
                        tb = work.tile([P, TQ], f32, tag="t1", name="tb")
                        nc.scalar.activation(tb[:], ps[:], AF.Identity,
                                             bias=bf2_sb[:, nt:nt + 1])
                        ob = work.tile([P, TQ], f32, tag="t2", name="ob")
                        nc.vector.tensor_add(ob[:], tb[:], h1[:, nt])
                        nc.sync.dma_start(outT.rearrange("(kt p) t -> p kt t", p=P)[:, nt], ob[:])

    nc.compile()
    return nc


# ---------- host-side constants ----------
def _bf16(a):
    import ml_dtypes
    return np.ascontiguousarray(a).astype(ml_dtypes.bfloat16)


def _f8(a, s=1.0):
    import ml_dtypes
    return np.ascontiguousarray(np.asarray(a) * s).astype(ml_dtypes.float8_e4m3)


def _rope_tables(t_idx, c):
    """cos/sin [len(t_idx), c] faithful to reference rope_apply."""
    freq = (t_idx.astype(np.float64) + 1.0)[:, None]
    pos = np.repeat(np.arange(c // 2, dtype=np.float64), 2)[None, :]
    theta = np.exp(-2.0 * pos / c * np.log(10000.0))
    ang = freq * theta
    return np.cos(ang).astype(np.float32), np.sin(ang).astype(np.float32)


def _host_consts():
    # rot matrix: y[m] = rot(x)[m] -> m even: -x[m+1]; m odd: x[m-1]
    rot = np.zeros((P, P), np.float32)
    for m in range(P):
        if m % 2 == 0:
            rot[m + 1, m] = -1.0
        else:
            rot[m - 1, m] = 1.0
    selpair = np.zeros((2, P), np.float32)
    selpair[0, 0:DK] = 1.0
    selpair[1, DK:P] = 1.0
    ones = np.ones((P, P), np.float32)
    epsc = np.full((P, 1), EPS, np.float32)
    return rot, selpair, ones, epsc


def _prep_inputs(inputs):
    """Build per-core in_maps from full inputs."""
    x = np.asarray(inputs["x"], np.float32)
    rms1 = np.asarray(inputs["rms1"], np.float32)
    rms2 = np.asarray(inputs["rms2"], np.float32)
    W_dkv = np.asarray(inputs["W_dkv"], np.float32) * rms1[:, None]
    b_dkv = np.asarray(inputs["b_dkv"], np.float32)
    W_kr = np.asarray(inputs["W_kr"], np.float32)
    b_kr = np.asarray(inputs["b_kr"], np.float32)
    W_qr = np.asarray(inputs["W_qr"], np.float32) * SCALE
    b_qr = np.asarray(inputs["b_qr"], np.float32) * SCALE
    W_kv = np.asarray(inputs["W_kv"], np.float32)
    b_kv = np.asarray(inputs["b_kv"], np.float32)
    W_q = np.asarray(inputs["W_q"], np.float32) * SCALE
    b_q = np.asarray(inputs["b_q"], np.float32) * SCALE
    W_o = np.asarray(inputs["W_o"], np.float32)
    b_o = np.asarray(inputs["b_o"], np.float32)
    b_v = b_kv[C:]
    b_o = b_o + b_v @ W_o          # v-bias folded: softmax(p)@(v+b) = p@v + b
    W_f1 = np.asarray(inputs["W_f1"], np.float32) * rms2[:, None]
    b_f1 = np.asarray(inputs["b_f1"], np.float32)
    W_f2 = np.asarray(inputs["W_f2"], np.float32)
    b_f2 = np.asarray(inputs["b_f2"], np.float32)

    rot, selpair, ones, epsc = _host_consts()

    def tiles4(w, nkt):  # [K, N] -> [N//P, P, nkt, P]; wt[nt,p,i,n] = w[i*P+p, nt*P+n]
        K, N = w.shape
        assert K == nkt * P
        return np.ascontiguousarray(w.reshape(nkt, P, N // P, P).transpose(2, 1, 0, 3))

    def bias_cols(b):  # [N] -> [P, N//P]
        return np.ascontiguousarray(b.reshape(-1, P).T)

    cosk, sink = _rope_tables(np.arange(T), DHR)          # [T, 64]

    shared = dict(
        w_dkv=_f8(tiles4(W_dkv, KT_C), S_W), b_dkv=bias_cols(b_dkv),
        w_kr=_f8(W_kr.reshape(KT_C, P, DHR).transpose(1, 0, 2), S_W),
        b_kr=b_kr[:, None].copy(),
        w_qr=_f8(tiles4(W_qr, KT_C), S_Q), b_qr=bias_cols(b_qr),
        w_kv_k=_f8(tiles4(W_kv[:, :C], KT_L), S_W),
        w_kv_v=_f8(
            W_kv[:, C:].reshape(KT_L, P, 2, 512).transpose(1, 2, 0, 3), S_W),
        b_k=bias_cols(b_kv[:C]),
        w_q=_f8(tiles4(W_q, KT_L), S_Q), b_q=bias_cols(b_q),
        w_o=_f8(tiles4(W_o, KT_C), S_W), b_o=bias_cols(b_o),
        w_f1=_bf16(tiles4(W_f1, KT_C).reshape(FF // P, P, KT_C * P)),
        b_f1=bias_cols(b_f1),
        w_f2=_bf16(tiles4(W_f2, KT_FF).reshape(C // P, P, KT_FF * P)),
        b_f2=bias_cols(b_f2),
        cosk=_bf16(cosk.T), sink=_bf16(sink.T),
        ones128=ones, rot128=rot, id128=_bf16(np.eye(P, dtype=np.float32)),
        selpair=selpair, epsc=epsc,
    )

    in_maps, sels = [], []
    for core in range(NCORES):
        b, par = divmod(core, 2)
        tb = np.array([2 * j + par for j in range(4)])
        selidx = (tb[:, None] * P + np.arange(P)[None, :]).reshape(-1)
        sels.append((b, selidx))
        cosq, sinq = _rope_tables(selidx, C)              # [TQ, C]
        cq = cosq.T.reshape(KT_C, P, TQ)
        sq = sinq.T.reshape(KT_C, P, TQ)
        # multiplicative diag-block mask: for key block kb the only partial
        # q-block is local jm = kb//2; 1.0 iff key_t <= q_t, duplicated over
        # the head-pair slab e
        am01 = np.zeros((NKB, P, 2, P), np.float32)
        for kb in range(NKB):
            jm = kb // 2
            qtok = selidx[jm * P:(jm + 1) * P][None, :]
            ktok = (kb * P + np.arange(P))[:, None]
            am01[kb, :, 0, :] = (ktok <= qtok)
            am01[kb, :, 1, :] = am01[kb, :, 0, :]
        m = dict(shared)
        m.update(
            xT_full=np.ascontiguousarray(x[b].T),
            xT_sel=np.ascontiguousarray(x[b][selidx].T),
            cosq=_bf16(cq), sinq=_bf16(sq),
            amask01=_f8(am01.reshape(NKB, P, 2 * P)),
        )
        in_maps.append(m)
    return in_maps, sels


def get_nc(repeat=None, skip=()):
    key = ("nc", repeat, tuple(skip))
    if key not in _CACHE:
        _CACHE[key] = _build_program(repeat, skip)
    return _CACHE[key]


def kernel(**inputs) -> np.ndarray:
    from concourse.bass_utils import run_bass_kernel_spmd
    nc = get_nc()
    in_maps, sels = _prep_inputs(inputs)
    results = run_bass_kernel_spmd(nc, in_maps, core_ids=list(range(NCORES))).results
    out = np.empty((B, T, C), np.float32)
    for core, (b, selidx) in enumerate(sels):
        out[b, selidx, :] = results[core]["outT"].T
    return out
